# revision 13
# baseline (speedup 1.0000x reference)
"""Bahdanau attention Trainium2 kernel.

reference:
    q = query @ Wq                      # (B, 1, H)
    k = keys @ Wk                       # (B, KLEN, H)
    feats = tanh(q + k)                 # (B, KLEN, H)
    scores = feats @ Wv                 # (B, KLEN, 1) -> (B, 1, KLEN)
    weights = softmax(scores, -1)
    context = weights @ keys            # (B, 1, H)
    returns (context, weights)

Sharding: data-parallel over batch, 4 batches per core on 8 cores.

Device data layout trick: the host ships keys twice in bf16 — once
transposed [H, KLEN] (for the d-contraction of the projection) and once
natural [KLEN, H] (for the i-contraction of the context) — which costs the
same HBM traffic as a single fp32 copy.

Per-core program, per 512-key chunk:
  - featsT[h, i] = tanh(sum_d Wk[d, h] keysT[d, i] + qproj[h])  (PE bf16
    matmuls into PSUM; ACT tanh with the per-partition bias = qprojT column)
  - scores[i] = sum_h Wv[h] featsT[h, i]   (PE, M=1 matmuls)
  - u = exp(scores) on ACT straight from PSUM; accum gives chunk sum(u).
    No max-subtraction is needed: |scores| <= ||Wv||_1 (tanh is bounded), far
    inside fp32 exp range.
  - u^T (128-partition column layout) via a small SB->SB scatter DMA.
  - context += u^T-column-weighted sums of natural keys (PE, M=1 matmuls,
    accumulated in PSUM across the whole batch).
Softmax normalisation (the division by sum u) happens on the host in fp64.
"""

import numpy as np

B, KLEN, H = 32, 4096, 512
NCORES = 8
BL = B // NCORES  # batches per core
NCH = KLEN // 1024  # i-chunks per batch (1024 keys each)

_PROG_CACHE = {}


def _split_multi_waits(nc, mybir, bass_rust):
    """walrus in this image accepts at most one sync-wait per instruction
    (two for EventSemaphore). Tile attaches several to one instruction;
    split the extras into standalone EventSemaphore instructions placed
    immediately before the owning instruction on the same engine."""
    ctr = 0
    for f in nc.m.functions:
        for blk in f.blocks:
            insts = blk.instructions
            new_list = []
            changed = False
            for inst in list(insts):
                si = inst.sync_info
                waits = list(si.on_wait) if si is not None and si.on_wait else []
                cap = 2 if isinstance(inst, mybir.InstEventSemaphore) else 1
                if len(waits) > cap:
                    extras, keep = waits[:-cap], waits[-cap:]
                    for j in range(0, len(extras), 2):
                        chunk = extras[j : j + 2]
                        ev = mybir.InstEventSemaphore(
                            name=f"WSPLIT-{ctr}", ins=[], outs=[]
                        )
                        ctr += 1
                        ev.engine = inst.engine
                        ev.sync_info = bass_rust.SyncInfo(on_wait=chunk, on_update=[])
                        nc.register_instruction(ev, overwrite=True)
                        new_list.append(ev)
                    si.on_wait = keep
                    inst.sync_info = si
                    changed = True
                new_list.append(inst)
            if changed:
                insts[:] = new_list
    return ctr


def _build():
    import bass_rust
    import concourse.bass as bass
    import concourse.mybir as mybir
    import concourse.tile as tile

    f32 = mybir.dt.float32
    f32r = mybir.dt.float32r
    bf16 = mybir.dt.bfloat16
    Tanh = mybir.ActivationFunctionType.Tanh
    Exp = mybir.ActivationFunctionType.Exp

    nc = bass.Bass("TRN2", target_bir_lowering=False, debug=False, num_devices=NCORES)

    keysT = nc.dram_tensor("keysT", [BL, H, KLEN], bf16, kind="ExternalInput")
    keysN = nc.dram_tensor("keysN", [BL, KLEN, H], bf16, kind="ExternalInput")
    query = nc.dram_tensor("query", [BL, 1, H], f32, kind="ExternalInput")
    Wq = nc.dram_tensor("Wq", [H, H], f32, kind="ExternalInput")
    Wk = nc.dram_tensor("Wk", [H, H], bf16, kind="ExternalInput")
    Wv = nc.dram_tensor("Wv", [H, 1], bf16, kind="ExternalInput")

    ctx_out = nc.dram_tensor("ctx_out", [BL, H], f32, kind="ExternalOutput")
    u_out = nc.dram_tensor("u_out", [BL, KLEN], f32, kind="ExternalOutput")
    su_out = nc.dram_tensor("su_out", [BL, 1], f32, kind="ExternalOutput")

    with tile.TileContext(nc) as tc:
        with (
            tc.tile_pool(name="const", bufs=1) as constp,
            tc.tile_pool(name="ktp", bufs=3) as ktp,
            tc.tile_pool(name="knp", bufs=2) as knp,
            tc.tile_pool(name="ftp", bufs=2) as ftp,
            tc.tile_pool(name="scp", bufs=2) as scp,
            tc.tile_pool(name="up", bufs=2) as up,
            tc.tile_pool(name="smallp", bufs=2) as smallp,
        ):
            # ---- constants ----
            wk_sb = constp.tile([128, 4, H], bf16)  # [d_in_chunk, dc, h]
            nc.sync.dma_start(
                out=wk_sb, in_=Wk.ap().rearrange("(dc p) h -> p dc h", p=128)
            )
            wq_sb = constp.tile([128, 4, H], f32)
            nc.sync.dma_start(
                out=wq_sb, in_=Wq.ap().rearrange("(dc p) h -> p dc h", p=128)
            )
            wv_sb = constp.tile([128, 4], bf16)  # [h_in_chunk, hc]
            nc.sync.dma_start(
                out=wv_sb, in_=Wv.ap().rearrange("(hc p) o -> p (hc o)", p=128)
            )
            qT_sb = constp.tile([128, 4, BL], f32)  # [d_in_chunk, dc, b]
            q_r = query.ap().rearrange("b one (dc p) -> dc p (b one)", p=128)
            for dc in range(4):
                nc.gpsimd.dma_start(out=qT_sb[:, dc, :], in_=q_r[dc])

            # ---- q projection: qprojT[h, b] (fp32) ----
            qprojT = constp.tile([128, 4, BL], f32)  # [h_in_chunk, hc, b]
            with tc.tile_pool(name="setup_ps", bufs=1, space="PSUM") as setup_ps:
                psq = setup_ps.tile([128, 4, BL], f32)
                for hc in range(4):
                    for dc in range(4):
                        nc.tensor.matmul(
                            psq[:, hc, :],
                            lhsT=wq_sb[:, dc, hc * 128 : (hc + 1) * 128],
                            rhs=qT_sb[:, dc, :],
                            start=(dc == 0),
                            stop=(dc == 3),
                        )
                nc.vector.tensor_copy(out=qprojT, in_=psq)

            with (
                tc.tile_pool(name="psF", bufs=2, space="PSUM") as psF_pool,
                tc.tile_pool(name="psS", bufs=1, space="PSUM") as psS_pool,
                tc.tile_pool(name="psC", bufs=2, space="PSUM") as psC_pool,
            ):
                for b in range(BL):
                    accs1 = smallp.tile([1, NCH], f32, tag="accs1")
                    u_b = up.tile([1, KLEN], f32, tag="u_b")
                    ubf = up.tile([1, KLEN], bf16, tag="ubf")
                    uT_bf = smallp.tile([128, 8 * NCH], bf16, tag="uT")
                    kn_b = knp.tile([128, NCH, 8, 512], bf16, tag="kn")
                    for ic in range(NCH):
                        # 1. load keysT chunk [d 512, i 1024] and natural chunk
                        ktT = ktp.tile([128, 4, 1024], bf16, tag="ktT")
                        eng1 = nc.sync if ic % 2 == 0 else nc.gpsimd
                        eng1.dma_start(
                            out=ktT,
                            in_=keysT.ap()[b, :, ic * 1024 : (ic + 1) * 1024].rearrange(
                                "(dc p) i -> p dc i", p=128
                            ),
                        )
                        eng2 = nc.gpsimd if ic % 2 == 0 else nc.sync
                        eng2.dma_start(
                            out=kn_b[:, ic, :, :],
                            in_=keysN.ap()[b, ic * 1024 : (ic + 1) * 1024, :].rearrange(
                                "(p s) d -> p s d", s=8
                            ),
                        )
                        # 2./3. projection (bf16, N=1024) + tanh(. + q)
                        ftT = ftp.tile([128, 4, 1024], bf16, tag="ftT")
                        for hc in range(4):
                            psF = psF_pool.tile([128, 2, 512], f32, tag="F")
                            for j in range(2):
                                for dc in range(4):
                                    nc.tensor.matmul(
                                        psF[:, j, :],
                                        lhsT=wk_sb[:, dc, hc * 128 : (hc + 1) * 128],
                                        rhs=ktT[:, dc, j * 512 : (j + 1) * 512],
                                        start=(dc == 0),
                                        stop=(dc == 3),
                                    )
                            nc.scalar.activation(
                                out=ftT[:, hc, :],
                                in_=psF[:, :, :].rearrange("p a b -> p (a b)"),
                                func=Tanh,
                                bias=qprojT[:, hc, b : b + 1],
                                scale=1.0,
                            )
                        # 4. scores: psS[0, i] = sum_h Wv[h] featsT[h, i]
                        psS = psS_pool.tile([1, 2, 512], f32, tag="S")
                        for hc in range(4):
                            for j in range(2):
                                nc.tensor.matmul(
                                    psS[0:1, j, :],
                                    lhsT=wv_sb[:, hc : hc + 1],
                                    rhs=ftT[:, hc, j * 512 : (j + 1) * 512],
                                    start=(hc == 0),
                                    stop=(hc == 3),
                                )
                        # 5. u = exp(scores) from PSUM; accum -> chunk sum(u)
                        nc.scalar.activation(
                            out=u_b[0:1, ic * 1024 : (ic + 1) * 1024],
                            in_=psS[0:1, :, :].rearrange("o a b -> o (a b)"),
                            func=Exp,
                            accum_out=accs1[0:1, ic : ic + 1],
                        )
                        # 6. bf16 copy of u for the transpose + context
                        nc.vector.tensor_copy(
                            out=ubf[0:1, ic * 1024 : (ic + 1) * 1024],
                            in_=u_b[0:1, ic * 1024 : (ic + 1) * 1024],
                        )
                        # 7. u^T columns via SB->SB scatter DMA
                        nc.sync.dma_start(
                            out=uT_bf[:, ic * 8 : (ic + 1) * 8],
                            in_=ubf[
                                0:1, ic * 1024 : (ic + 1) * 1024
                            ].rearrange("o (p s) -> o p s", s=8),
                        )
                    # ---- batch tail: context matmuls (inputs all ready,
                    # so the PE never stalls mid-chunk on the u scatter) ----
                    psC = psC_pool.tile([1, 512], f32, tag="C")
                    for ic in range(NCH):
                        for sb in range(8):
                            nc.tensor.matmul(
                                psC[0:1, :],
                                lhsT=uT_bf[:, ic * 8 + sb : ic * 8 + sb + 1],
                                rhs=kn_b[:, ic, sb, :],
                                start=(ic == 0 and sb == 0),
                                stop=(ic == NCH - 1 and sb == 7),
                            )
                    # ---- batch tail ----
                    ctxv = smallp.tile([1, 512], f32, tag="ctxv")
                    nc.vector.tensor_copy(out=ctxv, in_=psC)
                    su = smallp.tile([1, 1], f32, tag="su")
                    nc.vector.tensor_reduce(
                        out=su,
                        in_=accs1,
                        axis=mybir.AxisListType.X,
                        op=mybir.AluOpType.add,
                    )
                    nc.gpsimd.dma_start(out=ctx_out.ap()[b : b + 1, :], in_=ctxv)
                    nc.gpsimd.dma_start(out=su_out.ap()[b : b + 1, :], in_=su)
                    nc.gpsimd.dma_start(out=u_out.ap()[b : b + 1, :], in_=u_b)

    _split_multi_waits(nc, mybir, bass_rust)
    return nc


def _get_prog():
    if "nc" not in _PROG_CACHE:
        _PROG_CACHE["nc"] = _build()
    return _PROG_CACHE["nc"]


def kernel(query, keys, Wq, Wk, Wv):
    import ml_dtypes
    from concourse.bass_utils import run_bass_kernel_spmd

    bf16 = ml_dtypes.bfloat16

    query = np.ascontiguousarray(np.asarray(query, dtype=np.float32))
    keys = np.ascontiguousarray(np.asarray(keys, dtype=np.float32))
    Wq = np.ascontiguousarray(np.asarray(Wq, dtype=np.float32))
    Wk_bf = np.ascontiguousarray(np.asarray(Wk, dtype=np.float32).astype(bf16))
    Wv_bf = np.ascontiguousarray(np.asarray(Wv, dtype=np.float32).astype(bf16))

    nc = _get_prog()

    in_maps = []
    for c in range(NCORES):
        sl = slice(c * BL, (c + 1) * BL)
        kc = keys[sl]
        in_maps.append(
            {
                "keysT": np.ascontiguousarray(
                    kc.transpose(0, 2, 1).astype(bf16)
                ),  # [BL, H, KLEN]
                "keysN": np.ascontiguousarray(kc.astype(bf16)),  # [BL, KLEN, H]
                "query": np.ascontiguousarray(query[sl]),
                "Wq": Wq,
                "Wk": Wk_bf,
                "Wv": Wv_bf,
            }
        )

    res = run_bass_kernel_spmd(nc, in_maps, core_ids=list(range(NCORES)))

    context = np.empty((B, 1, H), dtype=np.float32)
    weights = np.empty((B, 1, KLEN), dtype=np.float32)
    for c in range(NCORES):
        r = res.results[c]
        ctx = r["ctx_out"].astype(np.float64)  # [BL, H]
        u = r["u_out"].astype(np.float64)  # [BL, KLEN]
        su = r["su_out"].astype(np.float64)  # [BL, 1]
        for b in range(BL):
            gb = c * BL + b
            context[gb, 0, :] = (ctx[b] / su[b, 0]).astype(np.float32)
            weights[gb, 0, :] = (u[b] / su[b, 0]).astype(np.float32)
    return context, weights


# revision 14
# speedup vs baseline: 1.0664x; 1.0664x over previous
"""Bahdanau attention Trainium2 kernel.

reference:
    q = query @ Wq                      # (B, 1, H)
    k = keys @ Wk                       # (B, KLEN, H)
    feats = tanh(q + k)                 # (B, KLEN, H)
    scores = feats @ Wv                 # (B, KLEN, 1) -> (B, 1, KLEN)
    weights = softmax(scores, -1)
    context = weights @ keys            # (B, 1, H)
    returns (context, weights)

Sharding: data-parallel over batch, 4 batches per core on 8 cores.

Device data layout trick: the host ships keys twice in bf16 — once
transposed [H, KLEN] (for the d-contraction of the projection) and once
natural [KLEN, H] (for the i-contraction of the context) — which costs the
same HBM traffic as a single fp32 copy.

Per-core program, per 512-key chunk:
  - featsT[h, i] = tanh(sum_d Wk[d, h] keysT[d, i] + qproj[h])  (PE bf16
    matmuls into PSUM; ACT tanh with the per-partition bias = qprojT column)
  - scores[i] = sum_h Wv[h] featsT[h, i]   (PE, M=1 matmuls)
  - u = exp(scores) on ACT straight from PSUM; accum gives chunk sum(u).
    No max-subtraction is needed: |scores| <= ||Wv||_1 (tanh is bounded), far
    inside fp32 exp range.
  - u^T (128-partition column layout) via a small SB->SB scatter DMA.
  - context += u^T-column-weighted sums of natural keys (PE, M=1 matmuls,
    accumulated in PSUM across the whole batch).
Softmax normalisation (the division by sum u) happens on the host in fp64.
"""

import numpy as np

B, KLEN, H = 32, 4096, 512
NCORES = 8
BL = B // NCORES  # batches per core
NCH = KLEN // 1024  # i-chunks per batch (1024 keys each)

_PROG_CACHE = {}


def _split_multi_waits(nc, mybir, bass_rust):
    """walrus in this image accepts at most one sync-wait per instruction
    (two for EventSemaphore). Tile attaches several to one instruction;
    split the extras into standalone EventSemaphore instructions placed
    immediately before the owning instruction on the same engine."""
    ctr = 0
    for f in nc.m.functions:
        for blk in f.blocks:
            insts = blk.instructions
            new_list = []
            changed = False
            for inst in list(insts):
                si = inst.sync_info
                waits = list(si.on_wait) if si is not None and si.on_wait else []
                cap = 2 if isinstance(inst, mybir.InstEventSemaphore) else 1
                if len(waits) > cap:
                    extras, keep = waits[:-cap], waits[-cap:]
                    for j in range(0, len(extras), 2):
                        chunk = extras[j : j + 2]
                        ev = mybir.InstEventSemaphore(
                            name=f"WSPLIT-{ctr}", ins=[], outs=[]
                        )
                        ctr += 1
                        ev.engine = inst.engine
                        ev.sync_info = bass_rust.SyncInfo(on_wait=chunk, on_update=[])
                        nc.register_instruction(ev, overwrite=True)
                        new_list.append(ev)
                    si.on_wait = keep
                    inst.sync_info = si
                    changed = True
                new_list.append(inst)
            if changed:
                insts[:] = new_list
    return ctr


def _build():
    import bass_rust
    import concourse.bass as bass
    import concourse.mybir as mybir
    import concourse.tile as tile
    from concourse.masks import make_identity

    f32 = mybir.dt.float32
    f32r = mybir.dt.float32r
    bf16 = mybir.dt.bfloat16
    Tanh = mybir.ActivationFunctionType.Tanh
    Exp = mybir.ActivationFunctionType.Exp

    nc = bass.Bass("TRN2", target_bir_lowering=False, debug=False, num_devices=NCORES)

    keysT = nc.dram_tensor("keysT", [BL, H, KLEN], bf16, kind="ExternalInput")
    keysN = nc.dram_tensor("keysN", [BL, KLEN, H], bf16, kind="ExternalInput")
    query = nc.dram_tensor("query", [BL, 1, H], f32, kind="ExternalInput")
    Wq = nc.dram_tensor("Wq", [H, H], f32, kind="ExternalInput")
    Wk = nc.dram_tensor("Wk", [H, H], bf16, kind="ExternalInput")
    Wv = nc.dram_tensor("Wv", [H, 1], bf16, kind="ExternalInput")

    ctx_out = nc.dram_tensor("ctx_out", [BL, H], f32, kind="ExternalOutput")
    u_out = nc.dram_tensor("u_out", [BL, KLEN], f32, kind="ExternalOutput")
    su_out = nc.dram_tensor("su_out", [BL, 1], f32, kind="ExternalOutput")

    with tile.TileContext(nc) as tc:
        with (
            tc.tile_pool(name="const", bufs=1) as constp,
            tc.tile_pool(name="ktp", bufs=3) as ktp,
            tc.tile_pool(name="knp", bufs=2) as knp,
            tc.tile_pool(name="ftp", bufs=2) as ftp,
            tc.tile_pool(name="scp", bufs=2) as scp,
            tc.tile_pool(name="up", bufs=2) as up,
            tc.tile_pool(name="smallp", bufs=2) as smallp,
        ):
            # ---- constants ----
            wk_sb = constp.tile([128, 4, H], bf16)  # [d_in_chunk, dc, h]
            nc.sync.dma_start(
                out=wk_sb, in_=Wk.ap().rearrange("(dc p) h -> p dc h", p=128)
            )
            wq_sb = constp.tile([128, 4, H], f32)
            nc.gpsimd.dma_start(
                out=wq_sb, in_=Wq.ap().rearrange("(dc p) h -> p dc h", p=128)
            )
            wv_sb = constp.tile([128, 4], bf16)  # [h_in_chunk, hc]
            nc.gpsimd.dma_start(
                out=wv_sb, in_=Wv.ap().rearrange("(hc p) o -> p (hc o)", p=128)
            )
            # warm the ACT table (tanh/exp share one set) while DMAs run
            warm = constp.tile([128, 1], f32)
            nc.vector.memset(warm, 0.0)
            nc.scalar.activation(out=warm, in_=warm, func=Tanh)
            # query natural load + on-chip transpose (a strided direct load
            # would need 512 tiny descriptors and stalls the pipeline start)
            qn = constp.tile([4, H], f32)
            nc.gpsimd.dma_start(out=qn, in_=query.ap()[:, 0, :])
            ident4 = constp.tile([4, 4], f32)
            make_identity(nc, ident4)
            qT_sb = constp.tile([128, 4, BL], f32)  # [d_in_chunk, dc, b]

            # ---- q projection: qprojT[h, b] (fp32) ----
            qprojT = constp.tile([128, 4, BL], f32)  # [h_in_chunk, hc, b]
            with tc.tile_pool(name="setup_ps", bufs=1, space="PSUM") as setup_ps:
                psqT = setup_ps.tile([128, 4, BL], f32, tag="qT")
                for dc in range(4):
                    nc.tensor.transpose(
                        out=psqT[:, dc, :],
                        in_=qn[0:4, dc * 128 : (dc + 1) * 128],
                        identity=ident4,
                    )
                nc.vector.tensor_copy(out=qT_sb, in_=psqT)
                psq = setup_ps.tile([128, 4, BL], f32, tag="qp")
                for hc in range(4):
                    for dc in range(4):
                        nc.tensor.matmul(
                            psq[:, hc, :],
                            lhsT=wq_sb[:, dc, hc * 128 : (hc + 1) * 128],
                            rhs=qT_sb[:, dc, :],
                            start=(dc == 0),
                            stop=(dc == 3),
                        )
                nc.vector.tensor_copy(out=qprojT, in_=psq)

            with (
                tc.tile_pool(name="psF", bufs=2, space="PSUM") as psF_pool,
                tc.tile_pool(name="psS", bufs=1, space="PSUM") as psS_pool,
                tc.tile_pool(name="psC", bufs=2, space="PSUM") as psC_pool,
            ):
                for b in range(BL):
                    accs1 = smallp.tile([1, NCH], f32, tag="accs1")
                    u_b = up.tile([1, KLEN], f32, tag="u_b")
                    ubf = up.tile([1, KLEN], bf16, tag="ubf")
                    uT_bf = smallp.tile([128, 8 * NCH], bf16, tag="uT")
                    kn_b = knp.tile([128, NCH, 8, 512], bf16, tag="kn")
                    for ic in range(NCH):
                        # 1. load keysT chunk [d 512, i 1024] and natural chunk
                        ktT = ktp.tile([128, 4, 1024], bf16, tag="ktT")
                        eng1 = nc.sync if ic % 2 == 0 else nc.gpsimd
                        eng1.dma_start(
                            out=ktT,
                            in_=keysT.ap()[b, :, ic * 1024 : (ic + 1) * 1024].rearrange(
                                "(dc p) i -> p dc i", p=128
                            ),
                        )
                        eng2 = nc.gpsimd if ic % 2 == 0 else nc.sync
                        eng2.dma_start(
                            out=kn_b[:, ic, :, :],
                            in_=keysN.ap()[b, ic * 1024 : (ic + 1) * 1024, :].rearrange(
                                "(p s) d -> p s d", s=8
                            ),
                        )
                        # 2./3. projection (bf16, N=1024) + tanh(. + q)
                        ftT = ftp.tile([128, 4, 1024], bf16, tag="ftT")
                        for hc in range(4):
                            psF = psF_pool.tile([128, 2, 512], f32, tag="F")
                            for j in range(2):
                                for dc in range(4):
                                    nc.tensor.matmul(
                                        psF[:, j, :],
                                        lhsT=wk_sb[:, dc, hc * 128 : (hc + 1) * 128],
                                        rhs=ktT[:, dc, j * 512 : (j + 1) * 512],
                                        start=(dc == 0),
                                        stop=(dc == 3),
                                    )
                            nc.scalar.activation(
                                out=ftT[:, hc, :],
                                in_=psF[:, :, :].rearrange("p a b -> p (a b)"),
                                func=Tanh,
                                bias=qprojT[:, hc, b : b + 1],
                                scale=1.0,
                            )
                        # 4. scores: psS[0, i] = sum_h Wv[h] featsT[h, i]
                        psS = psS_pool.tile([1, 2, 512], f32, tag="S")
                        for hc in range(4):
                            for j in range(2):
                                nc.tensor.matmul(
                                    psS[0:1, j, :],
                                    lhsT=wv_sb[:, hc : hc + 1],
                                    rhs=ftT[:, hc, j * 512 : (j + 1) * 512],
                                    start=(hc == 0),
                                    stop=(hc == 3),
                                )
                        # 5. u = exp(scores) from PSUM; accum -> chunk sum(u)
                        nc.scalar.activation(
                            out=u_b[0:1, ic * 1024 : (ic + 1) * 1024],
                            in_=psS[0:1, :, :].rearrange("o a b -> o (a b)"),
                            func=Exp,
                            accum_out=accs1[0:1, ic : ic + 1],
                        )
                        # 6. bf16 copy of u for the transpose + context
                        nc.vector.tensor_copy(
                            out=ubf[0:1, ic * 1024 : (ic + 1) * 1024],
                            in_=u_b[0:1, ic * 1024 : (ic + 1) * 1024],
                        )
                        # 7. u^T columns via SB->SB scatter DMA
                        nc.sync.dma_start(
                            out=uT_bf[:, ic * 8 : (ic + 1) * 8],
                            in_=ubf[
                                0:1, ic * 1024 : (ic + 1) * 1024
                            ].rearrange("o (p s) -> o p s", s=8),
                        )
                    # ---- batch tail: context matmuls (inputs all ready,
                    # so the PE never stalls mid-chunk on the u scatter) ----
                    psC = psC_pool.tile([1, 512], f32, tag="C")
                    for ic in range(NCH):
                        for sb in range(8):
                            nc.tensor.matmul(
                                psC[0:1, :],
                                lhsT=uT_bf[:, ic * 8 + sb : ic * 8 + sb + 1],
                                rhs=kn_b[:, ic, sb, :],
                                start=(ic == 0 and sb == 0),
                                stop=(ic == NCH - 1 and sb == 7),
                            )
                    # ---- batch tail ----
                    ctxv = smallp.tile([1, 512], f32, tag="ctxv")
                    nc.vector.tensor_copy(out=ctxv, in_=psC)
                    su = smallp.tile([1, 1], f32, tag="su")
                    nc.vector.tensor_reduce(
                        out=su,
                        in_=accs1,
                        axis=mybir.AxisListType.X,
                        op=mybir.AluOpType.add,
                    )
                    nc.gpsimd.dma_start(out=ctx_out.ap()[b : b + 1, :], in_=ctxv)
                    nc.gpsimd.dma_start(out=su_out.ap()[b : b + 1, :], in_=su)
                    nc.gpsimd.dma_start(out=u_out.ap()[b : b + 1, :], in_=u_b)

    _split_multi_waits(nc, mybir, bass_rust)
    return nc


def _get_prog():
    if "nc" not in _PROG_CACHE:
        _PROG_CACHE["nc"] = _build()
    return _PROG_CACHE["nc"]


def kernel(query, keys, Wq, Wk, Wv):
    import ml_dtypes
    from concourse.bass_utils import run_bass_kernel_spmd

    bf16 = ml_dtypes.bfloat16

    query = np.ascontiguousarray(np.asarray(query, dtype=np.float32))
    keys = np.ascontiguousarray(np.asarray(keys, dtype=np.float32))
    Wq = np.ascontiguousarray(np.asarray(Wq, dtype=np.float32))
    Wk_bf = np.ascontiguousarray(np.asarray(Wk, dtype=np.float32).astype(bf16))
    Wv_bf = np.ascontiguousarray(np.asarray(Wv, dtype=np.float32).astype(bf16))

    nc = _get_prog()

    in_maps = []
    for c in range(NCORES):
        sl = slice(c * BL, (c + 1) * BL)
        kc = keys[sl]
        in_maps.append(
            {
                "keysT": np.ascontiguousarray(
                    kc.transpose(0, 2, 1).astype(bf16)
                ),  # [BL, H, KLEN]
                "keysN": np.ascontiguousarray(kc.astype(bf16)),  # [BL, KLEN, H]
                "query": np.ascontiguousarray(query[sl]),
                "Wq": Wq,
                "Wk": Wk_bf,
                "Wv": Wv_bf,
            }
        )

    res = run_bass_kernel_spmd(nc, in_maps, core_ids=list(range(NCORES)))

    context = np.empty((B, 1, H), dtype=np.float32)
    weights = np.empty((B, 1, KLEN), dtype=np.float32)
    for c in range(NCORES):
        r = res.results[c]
        ctx = r["ctx_out"].astype(np.float64)  # [BL, H]
        u = r["u_out"].astype(np.float64)  # [BL, KLEN]
        su = r["su_out"].astype(np.float64)  # [BL, 1]
        for b in range(BL):
            gb = c * BL + b
            context[gb, 0, :] = (ctx[b] / su[b, 0]).astype(np.float32)
            weights[gb, 0, :] = (u[b] / su[b, 0]).astype(np.float32)
    return context, weights
